# revision 17
# baseline (speedup 1.0000x reference)
"""Trainium2 Bass kernel: single transformer block (MHA + FFN + 2xLN).

Sharding: data-parallel over tokens. 8 cores; cores 0-3 own batch 0,
cores 4-7 own batch 1; each core owns 1024 consecutive tokens of its
batch. QKV/FFN/LN are purely token-local; attention needs all K/V of
the batch, obtained with one AllGather over each 4-core group.

Layout strategy: activations are kept transposed ([feature, token]) so
every GEMM is out = lhsT.T @ rhs with contraction on partitions.
Weights (torch Linear [out,in]) are transposed once on the PE array.
Matmuls run as float32r (full-rate fp32 for free dim >= 256). Softmax
skips the max subtraction (scores are O(6) here; exp is safe in fp32);
the softmax denominator falls out of the ctx matmul via a ones column
appended to V. LayerNorm runs along the partition axis with
ones-vector matmul sums and PE row-broadcasts.
"""

import os
import sys

for _p in (
    "/opt/trn_rl_repo",
    "/root/.axon_site",
    "/root/.axon_site/_ro/trn_rl_repo",
    "/root/.axon_site/_ro/pypackages",
):
    if os.path.isdir(_p) and _p not in sys.path:
        sys.path.append(_p)

import numpy as np

import concourse.bass as bass
import concourse.mybir as mybir
import concourse.tile as tile
from concourse import bacc
from concourse.bass_utils import run_bass_kernel_spmd
from concourse.masks import make_identity

F32 = mybir.dt.float32
F32R = mybir.dt.float32r
AF = mybir.ActivationFunctionType
ALU = mybir.AluOpType

B, S, D = 2, 4096, 768
H, DK = 12, 64
DFF = 3072
NCORES = 8
GROUP = 4  # cores per batch
TOK = (B * S) // NCORES  # 1024 tokens per core
TCH = TOK // 128  # 8
DCH = D // 128  # 6
FCH = DFF // 128  # 24
KV = S  # kv length per batch
KCH = KV // 128  # 32
EPS = 1e-5
RG = [[0, 1, 2, 3], [4, 5, 6, 7]]

K_REGION = 128 * DCH * TOK  # floats in the K.T region of the kv bounce
VW = H * 65  # 780: v row width incl. per-head ones column
V_REGION = TCH * 128 * VW


def _emit_ln(tc, ps_big, ps_st, sb_tmp, y, g_sb, beta_sb, out):
    """LayerNorm along the partition (feature) axis of y [128, DCH, TOK]."""
    nc = tc.nc
    ones_p, ones_f = tc._ones_p, tc._ones_f
    st = ps_st.tile([1, TOK], F32, tag="st1", name="st")
    stq = ps_st.tile([1, TOK], F32, tag="st2", name="stq")
    for q in range(TOK // 512):
        qs = slice(q * 512, (q + 1) * 512)
        for j in range(DCH):
            nc.tensor.matmul(
                st[0:1, qs], ones_p[:], y[:, j, qs],
                start=(j == 0), stop=(j == DCH - 1), skip_group_check=True,
            )
    for j in range(DCH):
        sq = sb_tmp.tile([128, TOK], F32R, tag="ysq", name="sq")
        nc.scalar.activation(sq[:], y[:, j, :], AF.Square)
        for q in range(TOK // 512):
            qs = slice(q * 512, (q + 1) * 512)
            nc.tensor.matmul(
                stq[0:1, qs], ones_p[:], sq[:, qs],
                start=(j == 0), stop=(j == DCH - 1), skip_group_check=True,
            )
    mu = sb_tmp.tile([1, TOK], F32, tag="lnmu", name="mu")
    var = sb_tmp.tile([1, TOK], F32, tag="lnvar", name="var")
    rs = sb_tmp.tile([1, TOK], F32R, tag="lnrs", name="rs")
    brow = sb_tmp.tile([1, TOK], F32R, tag="lnbrow", name="brow")
    nc.scalar.activation(mu[:], st[0:1, :], AF.Copy, scale=1.0 / D)
    nc.scalar.activation(var[:], stq[0:1, :], AF.Copy, scale=1.0 / D)
    nc.vector.tensor_tensor(brow[:], mu[:], mu[:], ALU.mult)  # mu^2
    nc.vector.tensor_tensor(var[:], var[:], brow[:], ALU.subtract)
    nc.scalar.activation(var[:], var[:], AF.Sqrt, bias=tc._eps[:])  # sd
    with nc.allow_low_precision(reason="fp32r operand rounding"):
        nc.vector.reciprocal(rs[:], var[:])
    nc.vector.tensor_tensor(brow[:], mu[:], rs[:], ALU.mult)  # mu*rs
    bcA = ps_big.tile([128, TOK], F32, tag="big", name="bcA")
    bcB = ps_big.tile([128, TOK], F32, tag="big", name="bcB")
    for q in range(TOK // 512):
        qs = slice(q * 512, (q + 1) * 512)
        nc.tensor.matmul(bcA[:, qs], ones_f[:], rs[:, qs],
                         start=True, stop=True, skip_group_check=True)
        nc.tensor.matmul(bcB[:, qs], ones_f[:], brow[:, qs],
                         start=True, stop=True, skip_group_check=True)
    for j in range(DCH):
        t1 = sb_tmp.tile([128, TOK], F32, tag="lnt", name="t1")
        nc.vector.tensor_tensor(t1[:], y[:, j, :], bcA[:], ALU.mult)
        nc.vector.tensor_tensor(t1[:], t1[:], bcB[:], ALU.subtract)
        nc.scalar.activation(out[:, j, :], t1[:], AF.Identity,
                             bias=beta_sb[:, j : j + 1], scale=g_sb[:, j : j + 1])


def _transpose_w_to_sbuf(tc, ps_tp, stage_pool, w_ap, dest, n_out_ch, n_in_ch):
    """dest[:, j, i*128:(i+1)*128] = w[i-chunk, j-chunk].T for torch [out,in] w."""
    nc = tc.nc
    ident = tc._ident
    for i in range(n_out_ch):
        win = stage_pool.tile([128, n_in_ch * 128], F32, tag="win", name="win")
        nc.sync.dma_start(win[:], w_ap[i * 128 : (i + 1) * 128, :])
        for j in range(n_in_ch):
            tp = ps_tp.tile([128, 128], F32, tag="tp", name="tp")
            nc.tensor.transpose(tp[:], win[:, j * 128 : (j + 1) * 128], ident[:])
            nc.vector.tensor_copy(dest[:, j, i * 128 : (i + 1) * 128], tp[:])


def _emit_body(tc, t_in, t_out):
    nc = tc.nc

    x_ap = t_in["x_shard"]
    out_ap = t_out["out_shard"]

    with tc.tile_pool(name="const", bufs=1) as const, \
         tc.tile_pool(name="dram", bufs=1, space="DRAM") as dram, \
         tc.tile_pool(name="pAct", bufs=1) as pAct:

        ident = const.tile([128, 128], F32)
        make_identity(nc, ident[:])
        tc._ident = ident
        onesf32_p = const.tile([128, H], F32)
        nc.vector.memset(onesf32_p[:], 1.0)
        onesf32_r = const.tile([1, 128], F32)
        nc.vector.memset(onesf32_r[:], 1.0)
        ones_p = const.tile([128, 1], F32R)
        nc.vector.tensor_copy(ones_p[:], onesf32_p[:, 0:1])
        ones_f = const.tile([1, 128], F32R)
        nc.vector.tensor_copy(ones_f[:], onesf32_r[:])
        tc._onesf32_p = onesf32_p
        eps_sb = const.tile([1, 1], F32)
        nc.vector.memset(eps_sb[:], EPS)
        tc._ones_p, tc._ones_f, tc._eps = ones_p, ones_f, eps_sb

        def load_percol(name, n):  # 1D [n*128] -> [128, n] (feature-chunked)
            t = const.tile([128, n], F32, tag=f"pc_{name}", name=f"pc_{name}")
            nc.sync.dma_start(t[:], t_in[name].rearrange("(c p) -> p c", p=128))
            return t

        bq_sb = load_percol("bq", DCH)
        bk_sb = load_percol("bk", DCH)
        bo_sb = load_percol("bo", DCH)
        b1_sb = load_percol("b1", FCH)
        b2_sb = load_percol("b2", DCH)
        g1_sb = load_percol("g1", DCH)
        beta1_sb = load_percol("beta1", DCH)
        g2_sb = load_percol("g2", DCH)
        beta2_sb = load_percol("beta2", DCH)
        bv_row = const.tile([1, D], F32R)
        nc.sync.dma_start(bv_row[:], t_in["bv"].unsqueeze(0).bitcast(F32R))

        # DRAM scratch
        kv_in = dram.tile([K_REGION + V_REGION], F32R)
        kv_out = dram.tile([GROUP, K_REGION + V_REGION], F32R)
        w1t_d = dram.tile([FCH, DCH, 128, 128], F32R)  # [i, j, din, dff]
        w2t_d = dram.tile([FCH, DCH, 128, 128], F32R)  # [i, m, dff, dout]

        # Big activation tiles: three slots, reused across phases via tags.
        xT = pAct.tile([128, DCH, TOK], F32R, tag="slotA")       # phases A-C
        QT = pAct.tile([128, DCH, TOK], F32R, tag="slotB")       # phases A-B
        woT = pAct.tile([128, DCH, D], F32R, tag="slotWo")       # phases B-C

        # ================= Phases A-C under shared PSUM pools ==============
        with tc.tile_pool(name="ps_big", bufs=2, space="PSUM") as ps_big:
          with tc.tile_pool(name="ps_tp", bufs=2, space="PSUM") as ps_tp:

            # ===== Phase A: x transpose, QKV projections, kv scatter =======
            with tc.tile_pool(name="pA", bufs=1) as pA, \
                 tc.tile_pool(name="pA1", bufs=3) as pA1:
                for t in range(TCH):
                    xin = pA1.tile([128, D], F32, tag="xin", name="xin")
                    nc.sync.dma_start(xin[:], x_ap[t * 128 : (t + 1) * 128, :])
                    for j in range(DCH):
                        tp = ps_tp.tile([128, 128], F32, tag="tp", name="tp")
                        nc.tensor.transpose(tp[:], xin[:, j * 128 : (j + 1) * 128],
                                            ident[:])
                        nc.vector.tensor_copy(xT[:, j, t * 128 : (t + 1) * 128],
                                              tp[:])

                wT = {}
                for wname in ("wq", "wk", "wv"):
                    wT[wname] = pA.tile([128, DCH, D], F32R, tag=f"{wname}T",
                                        name=f"{wname}T")
                    _transpose_w_to_sbuf(tc, ps_tp, pA1, t_in[wname],
                                         wT[wname], DCH, DCH)

                # Q.T and K.T GEMMs: out [dout, tok]
                for wname, bias_sb, dest in (("wq", bq_sb, "Q"),
                                             ("wk", bk_sb, "K")):
                    for m in range(DCH):
                        pso = ps_big.tile([128, TOK], F32, tag="big", name="pso")
                        for q in range(TOK // 512):
                            qs = slice(q * 512, (q + 1) * 512)
                            for j in range(DCH):
                                nc.tensor.matmul(
                                    pso[:, qs],
                                    wT[wname][:, j, m * 128 : (m + 1) * 128],
                                    xT[:, j, qs],
                                    start=(j == 0), stop=(j == DCH - 1),
                                    skip_group_check=True,
                                )
                        if dest == "Q":
                            nc.scalar.activation(QT[:, m, :], pso[:], AF.Identity,
                                                 bias=bias_sb[:, m : m + 1])
                        else:
                            kt = pA1.tile([128, TOK], F32R, tag="ktev", name="kt")
                            nc.scalar.activation(kt[:], pso[:], AF.Identity,
                                                 bias=bias_sb[:, m : m + 1])
                            nc.sync.dma_start(
                                kv_in[0:K_REGION].rearrange(
                                    "(p c t) -> p c t", p=128, c=DCH)[:, m, :],
                                kt[:],
                            )

                # V in natural layout [tok, dout] with per-head ones col
                for t in range(TCH):
                    psv = ps_big.tile([128, TOK], F32, tag="big", name="psv")
                    for lo, hi in ((0, 512), (512, D)):
                        qs = slice(lo, hi)
                        for j in range(DCH):
                            nc.tensor.matmul(
                                psv[:, qs],
                                xT[:, j, t * 128 : (t + 1) * 128],
                                wT["wv"][:, j, qs],
                                start=(j == 0), stop=False,
                                skip_group_check=True,
                            )
                        nc.tensor.matmul(  # bias row: + ones.T @ bv
                            psv[:, qs], ones_f[:], bv_row[:, qs],
                            start=False, stop=True, skip_group_check=True,
                        )
                    vt = pA1.tile([128, VW], F32R, tag="vtev", name="vt")
                    nc.vector.tensor_copy(
                        vt[:].rearrange("p (h f) -> p h f", h=H)[:, :, 0:DK],
                        psv[:, 0:D].rearrange("p (h f) -> p h f", h=H),
                    )
                    nc.vector.tensor_copy(
                        vt[:].rearrange("p (h f) -> p h f", h=H)[:, :, DK : DK + 1],
                        onesf32_p[:].unsqueeze(2),
                    )
                    nc.sync.dma_start(
                        kv_in[K_REGION:].rearrange("(t p f) -> t p f",
                                                   t=TCH, p=128)[t],
                        vt[:],
                    )

            # ============ AllGather K/V within each batch group ============
            nc.gpsimd.collective_compute(
                "AllGather", ALU.bypass, replica_groups=RG,
                ins=[kv_in[:].opt()], outs=[kv_out[:].opt()],
            )

            # ===== Phase B: attention; pre-transpose w1, w2 to DRAM ========
            ctxT = pAct.tile([128, DCH, TOK], F32R, tag="slotC")  # phases B-C

            with tc.tile_pool(name="pB", bufs=2) as pB, \
                 tc.tile_pool(name="pBe", bufs=3) as pBe, \
                 tc.tile_pool(name="ps_ctx", bufs=1, space="PSUM") as ps_ctx:
                # wo transpose -> SBUF (consumed in phase C)
                _transpose_w_to_sbuf(tc, ps_tp, pB, t_in["wo"], woT, DCH, DCH)
                # w1 transpose -> DRAM scratch [i, j, din, dff]
                for i in range(FCH):
                    win = pB.tile([128, D], F32, tag="win", name="win")
                    nc.sync.dma_start(win[:],
                                      t_in["w1"][i * 128 : (i + 1) * 128, :])
                    for j in range(DCH):
                        tp = ps_tp.tile([128, 128], F32, tag="tp", name="tp")
                        nc.tensor.transpose(tp[:], win[:, j * 128 : (j + 1) * 128],
                                            ident[:])
                        wst = pB.tile([128, 128], F32R, tag="wst", name="wst")
                        nc.vector.tensor_copy(wst[:], tp[:])
                        nc.sync.dma_start(w1t_d[i, j], wst[:])
                # w2 transpose -> DRAM scratch [i, m, dff, dout]
                for m in range(DCH):
                    for quarter in range(4):
                        win = pB.tile([128, D], F32, tag="win", name="win")
                        cols = slice(quarter * D, (quarter + 1) * D)
                        nc.sync.dma_start(
                            win[:], t_in["w2"][m * 128 : (m + 1) * 128, cols])
                        for ii in range(DCH):
                            i = quarter * DCH + ii
                            tp = ps_tp.tile([128, 128], F32, tag="tp", name="tp")
                            nc.tensor.transpose(
                                tp[:], win[:, ii * 128 : (ii + 1) * 128], ident[:])
                            wst = pB.tile([128, 128], F32R, tag="wst", name="wst")
                            nc.vector.tensor_copy(wst[:], tp[:])
                            nc.sync.dma_start(w2t_d[i, m], wst[:])

                # attention, head by head
                for h in range(H):
                    plo = (h % 2) * 64
                    jch = h // 2
                    KhT = pB.tile([128, KV], F32R, tag="kh", name="KhT")
                    Vh = pB.tile([128, KCH, 65], F32R, tag="vh", name="Vh")
                    for r in range(GROUP):
                        nc.sync.dma_start(
                            KhT[plo : plo + 64, r * TOK : (r + 1) * TOK],
                            kv_out[r, 0:K_REGION].rearrange(
                                "(p c t) -> p c t", p=128, c=DCH
                            )[plo : plo + 64, jch, :],
                        )
                        nc.sync.dma_start(
                            Vh[:, r * TCH : (r + 1) * TCH, :],
                            kv_out[r, K_REGION:].rearrange(
                                "(t p f) -> p t f", t=TCH, p=128
                            )[:, :, h * 65 : (h + 1) * 65],
                        )
                    QhT = QT[plo : plo + 64, jch, :]
                    ps_acc = ps_ctx.tile([65, TOK], F32, tag="ctx", name="ps_acc")
                    for c in range(KCH):
                        ps_s = ps_big.tile([128, TOK], F32, tag="big", name="ps_s")
                        for q in range(TOK // 512):
                            qs = slice(q * 512, (q + 1) * 512)
                            nc.tensor.matmul(
                                ps_s[:, qs],
                                KhT[plo : plo + 64, c * 128 : (c + 1) * 128],
                                QhT[:, qs],
                                start=True, stop=True, skip_group_check=True,
                            )
                        E = pBe.tile([128, TOK], F32R, tag="E", name="E")
                        nc.scalar.activation(E[:], ps_s[:], AF.Exp,
                                             scale=1.0 / float(np.sqrt(DK)))
                        for q in range(TOK // 512):
                            qs = slice(q * 512, (q + 1) * 512)
                            nc.tensor.matmul(
                                ps_acc[:, qs], Vh[:, c, :], E[:, qs],
                                start=(c == 0), stop=(c == KCH - 1),
                                skip_group_check=True,
                            )
                    # normalize: ctx.T[f, q] * (1/denom[q])
                    rec = pBe.tile([1, TOK], F32R, tag="rec", name="rec")
                    with nc.allow_low_precision(reason="fp32r operand rounding"):
                        nc.vector.reciprocal(rec[:], ps_acc[64:65, :])
                    bc = ps_big.tile([128, TOK], F32, tag="big", name="bc")
                    for q in range(TOK // 512):
                        qs = slice(q * 512, (q + 1) * 512)
                        nc.tensor.matmul(bc[:, qs], ones_f[:], rec[:, qs],
                                         start=True, stop=True,
                                         skip_group_check=True)
                    bc_sb = pBe.tile([64, TOK], F32, tag="bc_sb", name="bc_sb")
                    nc.vector.tensor_copy(bc_sb[:], bc[0:64, :])
                    nc.vector.tensor_tensor(
                        ctxT[plo : plo + 64, jch, :], ps_acc[0:64, :],
                        bc_sb[:], ALU.mult,
                    )

          # ===== Phase C: O-projection + residual + LN1 ====================
          if True:
            n1 = pAct.tile([128, DCH, TOK], F32R, tag="slotB")  # reuses QT slot

            with tc.tile_pool(name="pC", bufs=1) as pC, \
                 tc.tile_pool(name="pC2", bufs=2) as pC2, \
                 tc.tile_pool(name="ps_st", bufs=1, space="PSUM") as ps_st:
                y1 = pC.tile([128, DCH, TOK], F32R, tag="y1", name="y1")
                for m in range(DCH):
                    pso = ps_big.tile([128, TOK], F32, tag="big", name="pso")
                    for q in range(TOK // 512):
                        qs = slice(q * 512, (q + 1) * 512)
                        for j in range(DCH):
                            nc.tensor.matmul(
                                pso[:, qs],
                                woT[:, j, m * 128 : (m + 1) * 128],
                                ctxT[:, j, qs],
                                start=(j == 0), stop=(j == DCH - 1),
                                skip_group_check=True,
                            )
                    nc.scalar.activation(y1[:, m, :], pso[:], AF.Identity,
                                         bias=bo_sb[:, m : m + 1])
                    nc.vector.tensor_tensor(y1[:, m, :], y1[:, m, :],
                                            xT[:, m, :], ALU.add)
                _emit_ln(tc, ps_big, ps_st, pC2, y1, g1_sb, beta1_sb, n1)

        # ================= Phase D: FFN (+ residual) =======================
        y2 = pAct.tile([128, DCH, TOK], F32R, tag="slotA")  # reuses xT slot
        with tc.tile_pool(name="ps_ffn", bufs=1, space="PSUM") as ps_ffn, \
             tc.tile_pool(name="ps_h", bufs=2, space="PSUM") as ps_h, \
             tc.tile_pool(name="pD", bufs=2) as pD, \
             tc.tile_pool(name="pDh", bufs=3) as pDh:
            for half in range(2):
                hs = slice(half * 512, (half + 1) * 512)
                ps2 = ps_ffn.tile([128, DCH, 512], F32, tag="ffn2", name="ps2")
                for i in range(FCH):
                    w1t = pD.tile([128, DCH, 128], F32R, tag="w1t", name="w1t")
                    nc.sync.dma_start(w1t[:],
                                      w1t_d[i].rearrange("j p f -> p j f"))
                    w2t = pD.tile([128, DCH, 128], F32R, tag="w2t", name="w2t")
                    nc.sync.dma_start(w2t[:],
                                      w2t_d[i].rearrange("m p f -> p m f"))
                    psh = ps_h.tile([128, 512], F32, tag="h", name="psh")
                    for j in range(DCH):
                        nc.tensor.matmul(
                            psh[:], w1t[:, j, :], n1[:, j, hs],
                            start=(j == 0), stop=(j == DCH - 1),
                            skip_group_check=True,
                        )
                    hsb = pDh.tile([128, 512], F32R, tag="hsb", name="hsb")
                    nc.scalar.activation(hsb[:], psh[:], AF.Gelu,
                                         bias=b1_sb[:, i : i + 1])
                    for m in range(DCH):
                        nc.tensor.matmul(
                            ps2[:, m, :], w2t[:, m, :], hsb[:],
                            start=(i == 0), stop=(i == FCH - 1),
                            skip_group_check=True,
                        )
                for m in range(DCH):
                    nc.scalar.activation(y2[:, m, hs], ps2[:, m, :], AF.Identity,
                                         bias=b2_sb[:, m : m + 1])
                    nc.vector.tensor_tensor(y2[:, m, hs], y2[:, m, hs],
                                            n1[:, m, hs], ALU.add)

        # ================= Phase E: LN2 + output transpose =================
        yf = pAct.tile([128, DCH, TOK], F32, tag="slotC")  # reuses ctxT slot
        with tc.tile_pool(name="pE2", bufs=2) as pE2, \
             tc.tile_pool(name="ps_big2", bufs=2, space="PSUM") as ps_big2:
            with tc.tile_pool(name="ps_st2", bufs=1, space="PSUM") as ps_st2:
                _emit_ln(tc, ps_big2, ps_st2, pE2, y2, g2_sb, beta2_sb, yf)
            with tc.tile_pool(name="ps_tp2", bufs=2, space="PSUM") as ps_tp2:
                for t in range(TCH):
                    on = pE2.tile([128, D], F32, tag="on", name="on")
                    for j in range(DCH):
                        tp = ps_tp2.tile([128, 128], F32, tag="tp", name="tp")
                        nc.tensor.transpose(tp[:],
                                            yf[:, j, t * 128 : (t + 1) * 128],
                                            ident[:])
                        nc.vector.tensor_copy(on[:, j * 128 : (j + 1) * 128],
                                              tp[:])
                    nc.sync.dma_start(out_ap[t * 128 : (t + 1) * 128, :], on[:])


_CACHE = {}


def _build():
    if "nc" in _CACHE:
        return _CACHE["nc"]
    nc = bacc.Bacc("TRN2", target_bir_lowering=False, debug=False,
                   num_devices=NCORES)
    t_in = {}
    t_in["x_shard"] = nc.dram_tensor("x_shard", [TOK, D], F32,
                                     kind="ExternalInput").ap()
    for name, shape in (
        ("wq", [D, D]), ("bq", [D]), ("wk", [D, D]), ("bk", [D]),
        ("wv", [D, D]), ("bv", [D]), ("wo", [D, D]), ("bo", [D]),
        ("w1", [DFF, D]), ("b1", [DFF]), ("w2", [D, DFF]), ("b2", [D]),
        ("g1", [D]), ("beta1", [D]), ("g2", [D]), ("beta2", [D]),
    ):
        t_in[name] = nc.dram_tensor(name, shape, F32, kind="ExternalInput").ap()
    t_out = {"out_shard": nc.dram_tensor("out_shard", [TOK, D], F32,
                                         kind="ExternalOutput").ap()}
    with tile.TileContext(nc) as tc:
        _emit_body(tc, t_in, t_out)
    nc.compile()
    _CACHE["nc"] = nc
    return nc


def _in_maps(inputs):
    f = lambda k: np.ascontiguousarray(np.asarray(inputs[k], dtype=np.float32))
    x = f("x")
    shared = {k: f(k) for k in inputs if k != "x"}
    maps = []
    for core in range(NCORES):
        g, r = divmod(core, GROUP)
        m = dict(shared)
        m["x_shard"] = np.ascontiguousarray(x[g, r * TOK : (r + 1) * TOK, :])
        maps.append(m)
    return maps


def kernel(**inputs):
    nc = _build()
    maps = _in_maps(inputs)
    res = run_bass_kernel_spmd(nc, maps, core_ids=list(range(NCORES)))
    shards = [res.results[i]["out_shard"] for i in range(NCORES)]
    out = np.concatenate(shards, axis=0).reshape(B, S, D)
    return out.astype(np.float32)


# revision 18
# speedup vs baseline: 1.1236x; 1.1236x over previous
"""Trainium2 Bass kernel: single transformer block (MHA + FFN + 2xLN).

Sharding: data-parallel over tokens. 8 cores; cores 0-3 own batch 0,
cores 4-7 own batch 1; each core owns 1024 consecutive tokens of its
batch. QKV/FFN/LN are purely token-local; attention needs all K/V of
the batch, obtained with one AllGather over each 4-core group.

Layout strategy: activations are kept transposed ([feature, token]) so
every GEMM is out = lhsT.T @ rhs with contraction on partitions.
Weights (torch Linear [out,in]) are transposed once on the PE array.
Matmuls run as float32r (full-rate fp32 for free dim >= 256). Softmax
skips the max subtraction (scores are O(6) here; exp is safe in fp32);
the softmax denominator falls out of the ctx matmul via a ones column
appended to V. LayerNorm runs along the partition axis with
ones-vector matmul sums and PE row-broadcasts.
"""

import os
import sys

for _p in (
    "/opt/trn_rl_repo",
    "/root/.axon_site",
    "/root/.axon_site/_ro/trn_rl_repo",
    "/root/.axon_site/_ro/pypackages",
):
    if os.path.isdir(_p) and _p not in sys.path:
        sys.path.append(_p)

import numpy as np

import concourse.bass as bass
import concourse.mybir as mybir
import concourse.tile as tile
from concourse import bacc
from concourse.bass_utils import run_bass_kernel_spmd
from concourse.masks import make_identity

F32 = mybir.dt.float32
F32R = mybir.dt.float32r
AF = mybir.ActivationFunctionType
ALU = mybir.AluOpType

B, S, D = 2, 4096, 768
H, DK = 12, 64
DFF = 3072
NCORES = 8
GROUP = 4  # cores per batch
TOK = (B * S) // NCORES  # 1024 tokens per core
TCH = TOK // 128  # 8
DCH = D // 128  # 6
FCH = DFF // 128  # 24
KV = S  # kv length per batch
KCH = KV // 128  # 32
EPS = 1e-5
RG = [[0, 1, 2, 3], [4, 5, 6, 7]]

VW = H * 65  # 780: v row width incl. per-head ones column
NG = 3  # number of pipelined sub-gathers
HPG = H // NG  # heads per sub-gather (4)
CPG = HPG // 2  # K.T 128-row chunks per sub-gather (2)
KG_REGION = 128 * CPG * TOK  # floats of K.T per sub-gather
VG_REGION = TCH * 128 * (HPG * 65)  # floats of V per sub-gather


def _emit_ln(tc, ps_big, ps_st, sb_tmp, y, g_sb, beta_sb, out):
    """LayerNorm along the partition (feature) axis of y [128, DCH, TOK]."""
    nc = tc.nc
    ones_p, ones_f = tc._ones_p, tc._ones_f
    st = ps_st.tile([1, TOK], F32, tag="st1", name="st")
    stq = ps_st.tile([1, TOK], F32, tag="st2", name="stq")
    for q in range(TOK // 512):
        qs = slice(q * 512, (q + 1) * 512)
        for j in range(DCH):
            nc.tensor.matmul(
                st[0:1, qs], ones_p[:], y[:, j, qs],
                start=(j == 0), stop=(j == DCH - 1), skip_group_check=True,
            )
    for j in range(DCH):
        sq = sb_tmp.tile([128, TOK], F32R, tag="ysq", name="sq")
        nc.scalar.activation(sq[:], y[:, j, :], AF.Square)
        for q in range(TOK // 512):
            qs = slice(q * 512, (q + 1) * 512)
            nc.tensor.matmul(
                stq[0:1, qs], ones_p[:], sq[:, qs],
                start=(j == 0), stop=(j == DCH - 1), skip_group_check=True,
            )
    mu = sb_tmp.tile([1, TOK], F32, tag="lnmu", name="mu")
    var = sb_tmp.tile([1, TOK], F32, tag="lnvar", name="var")
    rs = sb_tmp.tile([1, TOK], F32R, tag="lnrs", name="rs")
    brow = sb_tmp.tile([1, TOK], F32R, tag="lnbrow", name="brow")
    nc.scalar.activation(mu[:], st[0:1, :], AF.Copy, scale=1.0 / D)
    nc.scalar.activation(var[:], stq[0:1, :], AF.Copy, scale=1.0 / D)
    nc.vector.tensor_tensor(brow[:], mu[:], mu[:], ALU.mult)  # mu^2
    nc.vector.tensor_tensor(var[:], var[:], brow[:], ALU.subtract)
    nc.scalar.activation(var[:], var[:], AF.Sqrt, bias=tc._eps[:])  # sd
    with nc.allow_low_precision(reason="fp32r operand rounding"):
        nc.vector.reciprocal(rs[:], var[:])
    nc.vector.tensor_tensor(brow[:], mu[:], rs[:], ALU.mult)  # mu*rs
    bcA = ps_big.tile([128, TOK], F32, tag="big", name="bcA")
    bcB = ps_big.tile([128, TOK], F32, tag="big", name="bcB")
    for q in range(TOK // 512):
        qs = slice(q * 512, (q + 1) * 512)
        nc.tensor.matmul(bcA[:, qs], ones_f[:], rs[:, qs],
                         start=True, stop=True, skip_group_check=True)
        nc.tensor.matmul(bcB[:, qs], ones_f[:], brow[:, qs],
                         start=True, stop=True, skip_group_check=True)
    for j in range(DCH):
        t1 = sb_tmp.tile([128, TOK], F32, tag="lnt", name="t1")
        nc.vector.tensor_tensor(t1[:], y[:, j, :], bcA[:], ALU.mult)
        nc.vector.tensor_tensor(t1[:], t1[:], bcB[:], ALU.subtract)
        nc.scalar.activation(out[:, j, :], t1[:], AF.Identity,
                             bias=beta_sb[:, j : j + 1], scale=g_sb[:, j : j + 1])


def _transpose_w_to_sbuf(tc, ps_tp, stage_pool, w_ap, dest, n_out_ch, n_in_ch):
    """dest[:, j, i*128:(i+1)*128] = w[i-chunk, j-chunk].T for torch [out,in] w."""
    nc = tc.nc
    ident = tc._ident
    for i in range(n_out_ch):
        win = stage_pool.tile([128, n_in_ch * 128], F32, tag="win", name="win")
        nc.sync.dma_start(win[:], w_ap[i * 128 : (i + 1) * 128, :])
        for j in range(n_in_ch):
            tp = ps_tp.tile([128, 128], F32, tag="tp", name="tp")
            nc.tensor.transpose(tp[:], win[:, j * 128 : (j + 1) * 128], ident[:])
            nc.vector.tensor_copy(dest[:, j, i * 128 : (i + 1) * 128], tp[:])


def _emit_body(tc, t_in, t_out):
    nc = tc.nc

    x_ap = t_in["x_shard"]
    out_ap = t_out["out_shard"]

    with tc.tile_pool(name="const", bufs=1) as const, \
         tc.tile_pool(name="dram", bufs=1, space="DRAM") as dram, \
         tc.tile_pool(name="pAct", bufs=1) as pAct:

        ident = const.tile([128, 128], F32)
        make_identity(nc, ident[:])
        tc._ident = ident
        onesf32_p = const.tile([128, H], F32)
        nc.vector.memset(onesf32_p[:], 1.0)
        onesf32_r = const.tile([1, 128], F32)
        nc.vector.memset(onesf32_r[:], 1.0)
        ones_p = const.tile([128, 1], F32R)
        nc.vector.tensor_copy(ones_p[:], onesf32_p[:, 0:1])
        ones_f = const.tile([1, 128], F32R)
        nc.vector.tensor_copy(ones_f[:], onesf32_r[:])
        tc._onesf32_p = onesf32_p
        eps_sb = const.tile([1, 1], F32)
        nc.vector.memset(eps_sb[:], EPS)
        tc._ones_p, tc._ones_f, tc._eps = ones_p, ones_f, eps_sb

        def load_percol(name, n):  # 1D [n*128] -> [128, n] (feature-chunked)
            t = const.tile([128, n], F32, tag=f"pc_{name}", name=f"pc_{name}")
            nc.sync.dma_start(t[:], t_in[name].rearrange("(c p) -> p c", p=128))
            return t

        bq_sb = load_percol("bq", DCH)
        bk_sb = load_percol("bk", DCH)
        bo_sb = load_percol("bo", DCH)
        b1_sb = load_percol("b1", FCH)
        b2_sb = load_percol("b2", DCH)
        g1_sb = load_percol("g1", DCH)
        beta1_sb = load_percol("beta1", DCH)
        g2_sb = load_percol("g2", DCH)
        beta2_sb = load_percol("beta2", DCH)
        bv_row = const.tile([1, D], F32R)
        nc.sync.dma_start(bv_row[:], t_in["bv"].unsqueeze(0).bitcast(F32R))

        # DRAM scratch
        kv_ins = [dram.tile([KG_REGION + VG_REGION], F32R, tag=f"kvi{g}",
                            name=f"kv_in{g}") for g in range(NG)]
        kv_outs = [dram.tile([GROUP, KG_REGION + VG_REGION], F32R,
                             tag=f"kvo{g}", name=f"kv_out{g}")
                   for g in range(NG)]
        w1t_d = dram.tile([FCH, DCH, 128, 128], F32R)  # [i, j, din, dff]
        w2t_d = dram.tile([FCH, DCH, 128, 128], F32R)  # [i, m, dff, dout]

        # Big activation tiles: three slots, reused across phases via tags.
        xT = pAct.tile([128, DCH, TOK], F32R, tag="slotA")       # phases A-C
        QT = pAct.tile([128, DCH, TOK], F32R, tag="slotB")       # phases A-B
        woT = pAct.tile([128, DCH, D], F32R, tag="slotWo")       # phases B-C

        # ================= Phases A-C under shared PSUM pools ==============
        with tc.tile_pool(name="ps_big", bufs=2, space="PSUM") as ps_big:
          with tc.tile_pool(name="ps_tp", bufs=2, space="PSUM") as ps_tp:

            # ===== Phase A: x transpose, QKV projections, kv scatter =======
            with tc.tile_pool(name="pA", bufs=1) as pA, \
                 tc.tile_pool(name="pA1", bufs=3) as pA1:
                for t in range(TCH):
                    xin = pA1.tile([128, D], F32, tag="xin", name="xin")
                    nc.sync.dma_start(xin[:], x_ap[t * 128 : (t + 1) * 128, :])
                    for j in range(DCH):
                        tp = ps_tp.tile([128, 128], F32, tag="tp", name="tp")
                        nc.tensor.transpose(tp[:], xin[:, j * 128 : (j + 1) * 128],
                                            ident[:])
                        nc.vector.tensor_copy(xT[:, j, t * 128 : (t + 1) * 128],
                                              tp[:])

                wT = {}
                for wname in ("wv", "wk", "wq"):
                    wT[wname] = pA.tile([128, DCH, D], F32R, tag=f"{wname}T",
                                        name=f"{wname}T")

                # V first (in natural layout [tok, dout] with per-head ones
                # col), so the K sub-gathers can launch as soon as each K.T
                # chunk pair is done.
                _transpose_w_to_sbuf(tc, ps_tp, pA1, t_in["wv"], wT["wv"],
                                     DCH, DCH)
                for t in range(TCH):
                    psv = ps_big.tile([128, TOK], F32, tag="big", name="psv")
                    for lo, hi in ((0, 512), (512, D)):
                        qs = slice(lo, hi)
                        for j in range(DCH):
                            nc.tensor.matmul(
                                psv[:, qs],
                                xT[:, j, t * 128 : (t + 1) * 128],
                                wT["wv"][:, j, qs],
                                start=(j == 0), stop=False,
                                skip_group_check=True,
                            )
                        nc.tensor.matmul(  # bias row: + ones.T @ bv
                            psv[:, qs], ones_f[:], bv_row[:, qs],
                            start=False, stop=True, skip_group_check=True,
                        )
                    vt = pA1.tile([128, VW], F32R, tag="vtev", name="vt")
                    nc.vector.tensor_copy(
                        vt[:].rearrange("p (h f) -> p h f", h=H)[:, :, 0:DK],
                        psv[:, 0:D].rearrange("p (h f) -> p h f", h=H),
                    )
                    nc.vector.tensor_copy(
                        vt[:].rearrange("p (h f) -> p h f", h=H)[:, :, DK : DK + 1],
                        onesf32_p[:].unsqueeze(2),
                    )
                    for g in range(NG):
                        nc.sync.dma_start(
                            kv_ins[g][KG_REGION:].rearrange(
                                "(t p f) -> t p f", t=TCH, p=128)[t],
                            vt[:, g * HPG * 65 : (g + 1) * HPG * 65],
                        )

                # K.T GEMMs; launch sub-gather g once its chunk pair is in
                _transpose_w_to_sbuf(tc, ps_tp, pA1, t_in["wk"], wT["wk"],
                                     DCH, DCH)
                for m in range(DCH):
                    pso = ps_big.tile([128, TOK], F32, tag="big", name="pso")
                    for q in range(TOK // 512):
                        qs = slice(q * 512, (q + 1) * 512)
                        for j in range(DCH):
                            nc.tensor.matmul(
                                pso[:, qs],
                                wT["wk"][:, j, m * 128 : (m + 1) * 128],
                                xT[:, j, qs],
                                start=(j == 0), stop=(j == DCH - 1),
                                skip_group_check=True,
                            )
                    kt = pA1.tile([128, TOK], F32R, tag="ktev", name="kt")
                    nc.scalar.activation(kt[:], pso[:], AF.Identity,
                                         bias=bk_sb[:, m : m + 1])
                    g = m // CPG
                    nc.sync.dma_start(
                        kv_ins[g][0:KG_REGION].rearrange(
                            "(p c t) -> p c t", p=128, c=CPG)[:, m % CPG, :],
                        kt[:],
                    )
                    if m % CPG == CPG - 1:
                        nc.gpsimd.collective_compute(
                            "AllGather", ALU.bypass, replica_groups=RG,
                            ins=[kv_ins[g][:].opt()],
                            outs=[kv_outs[g][:].opt()],
                        )

                # Q.T GEMMs (only needed once attention starts)
                _transpose_w_to_sbuf(tc, ps_tp, pA1, t_in["wq"], wT["wq"],
                                     DCH, DCH)
                for m in range(DCH):
                    pso = ps_big.tile([128, TOK], F32, tag="big", name="pso")
                    for q in range(TOK // 512):
                        qs = slice(q * 512, (q + 1) * 512)
                        for j in range(DCH):
                            nc.tensor.matmul(
                                pso[:, qs],
                                wT["wq"][:, j, m * 128 : (m + 1) * 128],
                                xT[:, j, qs],
                                start=(j == 0), stop=(j == DCH - 1),
                                skip_group_check=True,
                            )
                    nc.scalar.activation(QT[:, m, :], pso[:], AF.Identity,
                                         bias=bq_sb[:, m : m + 1])

            # ===== Phase B: attention; pre-transpose w1, w2 to DRAM ========
            ctxT = pAct.tile([128, DCH, TOK], F32R, tag="slotC")  # phases B-C

            with tc.tile_pool(name="pB", bufs=2) as pB, \
                 tc.tile_pool(name="pBe", bufs=3) as pBe, \
                 tc.tile_pool(name="ps_ctx", bufs=1, space="PSUM") as ps_ctx:
                # wo transpose -> SBUF (consumed in phase C)
                _transpose_w_to_sbuf(tc, ps_tp, pB, t_in["wo"], woT, DCH, DCH)
                # w1 transpose -> DRAM scratch [i, j, din, dff]
                for i in range(FCH):
                    win = pB.tile([128, D], F32, tag="win", name="win")
                    nc.sync.dma_start(win[:],
                                      t_in["w1"][i * 128 : (i + 1) * 128, :])
                    for j in range(DCH):
                        tp = ps_tp.tile([128, 128], F32, tag="tp", name="tp")
                        nc.tensor.transpose(tp[:], win[:, j * 128 : (j + 1) * 128],
                                            ident[:])
                        wst = pB.tile([128, 128], F32R, tag="wst", name="wst")
                        nc.vector.tensor_copy(wst[:], tp[:])
                        nc.sync.dma_start(w1t_d[i, j], wst[:])
                # w2 transpose -> DRAM scratch [i, m, dff, dout]
                for m in range(DCH):
                    for quarter in range(4):
                        win = pB.tile([128, D], F32, tag="win", name="win")
                        cols = slice(quarter * D, (quarter + 1) * D)
                        nc.sync.dma_start(
                            win[:], t_in["w2"][m * 128 : (m + 1) * 128, cols])
                        for ii in range(DCH):
                            i = quarter * DCH + ii
                            tp = ps_tp.tile([128, 128], F32, tag="tp", name="tp")
                            nc.tensor.transpose(
                                tp[:], win[:, ii * 128 : (ii + 1) * 128], ident[:])
                            wst = pB.tile([128, 128], F32R, tag="wst", name="wst")
                            nc.vector.tensor_copy(wst[:], tp[:])
                            nc.sync.dma_start(w2t_d[i, m], wst[:])

                # attention, head by head
                for h in range(H):
                    plo = (h % 2) * 64
                    jch = h // 2
                    kvo = kv_outs[h // HPG]
                    cc = jch % CPG
                    hh = h % HPG
                    KhT = pB.tile([128, KV], F32R, tag="kh", name="KhT")
                    Vh = pB.tile([128, KCH, 65], F32R, tag="vh", name="Vh")
                    for r in range(GROUP):
                        nc.sync.dma_start(
                            KhT[plo : plo + 64, r * TOK : (r + 1) * TOK],
                            kvo[r, 0:KG_REGION].rearrange(
                                "(p c t) -> p c t", p=128, c=CPG
                            )[plo : plo + 64, cc, :],
                        )
                        nc.sync.dma_start(
                            Vh[:, r * TCH : (r + 1) * TCH, :],
                            kvo[r, KG_REGION:].rearrange(
                                "(t p f) -> p t f", t=TCH, p=128
                            )[:, :, hh * 65 : (hh + 1) * 65],
                        )
                    QhT = QT[plo : plo + 64, jch, :]
                    ps_acc = ps_ctx.tile([65, TOK], F32, tag="ctx", name="ps_acc")
                    for c in range(KCH):
                        ps_s = ps_big.tile([128, TOK], F32, tag="big", name="ps_s")
                        for q in range(TOK // 512):
                            qs = slice(q * 512, (q + 1) * 512)
                            nc.tensor.matmul(
                                ps_s[:, qs],
                                KhT[plo : plo + 64, c * 128 : (c + 1) * 128],
                                QhT[:, qs],
                                start=True, stop=True, skip_group_check=True,
                            )
                        E = pBe.tile([128, TOK], F32R, tag="E", name="E")
                        nc.scalar.activation(E[:], ps_s[:], AF.Exp,
                                             scale=1.0 / float(np.sqrt(DK)))
                        for q in range(TOK // 512):
                            qs = slice(q * 512, (q + 1) * 512)
                            nc.tensor.matmul(
                                ps_acc[:, qs], Vh[:, c, :], E[:, qs],
                                start=(c == 0), stop=(c == KCH - 1),
                                skip_group_check=True,
                            )
                    # normalize: ctx.T[f, q] * (1/denom[q])
                    rec = pBe.tile([1, TOK], F32R, tag="rec", name="rec")
                    with nc.allow_low_precision(reason="fp32r operand rounding"):
                        nc.vector.reciprocal(rec[:], ps_acc[64:65, :])
                    bc = ps_big.tile([128, TOK], F32, tag="big", name="bc")
                    for q in range(TOK // 512):
                        qs = slice(q * 512, (q + 1) * 512)
                        nc.tensor.matmul(bc[:, qs], ones_f[:], rec[:, qs],
                                         start=True, stop=True,
                                         skip_group_check=True)
                    bc_sb = pBe.tile([64, TOK], F32, tag="bc_sb", name="bc_sb")
                    nc.vector.tensor_copy(bc_sb[:], bc[0:64, :])
                    nc.vector.tensor_tensor(
                        ctxT[plo : plo + 64, jch, :], ps_acc[0:64, :],
                        bc_sb[:], ALU.mult,
                    )

          # ===== Phase C: O-projection + residual + LN1 ====================
          if True:
            n1 = pAct.tile([128, DCH, TOK], F32R, tag="slotB")  # reuses QT slot

            with tc.tile_pool(name="pC", bufs=1) as pC, \
                 tc.tile_pool(name="pC2", bufs=2) as pC2, \
                 tc.tile_pool(name="ps_st", bufs=1, space="PSUM") as ps_st:
                y1 = pC.tile([128, DCH, TOK], F32R, tag="y1", name="y1")
                for m in range(DCH):
                    pso = ps_big.tile([128, TOK], F32, tag="big", name="pso")
                    for q in range(TOK // 512):
                        qs = slice(q * 512, (q + 1) * 512)
                        for j in range(DCH):
                            nc.tensor.matmul(
                                pso[:, qs],
                                woT[:, j, m * 128 : (m + 1) * 128],
                                ctxT[:, j, qs],
                                start=(j == 0), stop=(j == DCH - 1),
                                skip_group_check=True,
                            )
                    nc.scalar.activation(y1[:, m, :], pso[:], AF.Identity,
                                         bias=bo_sb[:, m : m + 1])
                    nc.vector.tensor_tensor(y1[:, m, :], y1[:, m, :],
                                            xT[:, m, :], ALU.add)
                _emit_ln(tc, ps_big, ps_st, pC2, y1, g1_sb, beta1_sb, n1)

        # ================= Phase D: FFN (+ residual) =======================
        y2 = pAct.tile([128, DCH, TOK], F32R, tag="slotA")  # reuses xT slot
        with tc.tile_pool(name="ps_ffn", bufs=1, space="PSUM") as ps_ffn, \
             tc.tile_pool(name="ps_h", bufs=2, space="PSUM") as ps_h, \
             tc.tile_pool(name="pD", bufs=4) as pD, \
             tc.tile_pool(name="pDh", bufs=3) as pDh:
            for half in range(2):
                hs = slice(half * 512, (half + 1) * 512)
                ps2 = ps_ffn.tile([128, DCH, 512], F32, tag="ffn2", name="ps2")
                for i in range(FCH):
                    w1t = pD.tile([128, DCH, 128], F32R, tag="w1t", name="w1t")
                    nc.sync.dma_start(w1t[:],
                                      w1t_d[i].rearrange("j p f -> p j f"))
                    w2t = pD.tile([128, DCH, 128], F32R, tag="w2t", name="w2t")
                    nc.sync.dma_start(w2t[:],
                                      w2t_d[i].rearrange("m p f -> p m f"))
                    psh = ps_h.tile([128, 512], F32, tag="h", name="psh")
                    for j in range(DCH):
                        nc.tensor.matmul(
                            psh[:], w1t[:, j, :], n1[:, j, hs],
                            start=(j == 0), stop=(j == DCH - 1),
                            skip_group_check=True,
                        )
                    hsb = pDh.tile([128, 512], F32R, tag="hsb", name="hsb")
                    nc.scalar.activation(hsb[:], psh[:], AF.Gelu,
                                         bias=b1_sb[:, i : i + 1])
                    for m in range(DCH):
                        nc.tensor.matmul(
                            ps2[:, m, :], w2t[:, m, :], hsb[:],
                            start=(i == 0), stop=(i == FCH - 1),
                            skip_group_check=True,
                        )
                for m in range(DCH):
                    nc.scalar.activation(y2[:, m, hs], ps2[:, m, :], AF.Identity,
                                         bias=b2_sb[:, m : m + 1])
                    nc.vector.tensor_tensor(y2[:, m, hs], y2[:, m, hs],
                                            n1[:, m, hs], ALU.add)

        # ================= Phase E: LN2 + output transpose =================
        yf = pAct.tile([128, DCH, TOK], F32, tag="slotC")  # reuses ctxT slot
        with tc.tile_pool(name="pE2", bufs=2) as pE2, \
             tc.tile_pool(name="ps_big2", bufs=2, space="PSUM") as ps_big2:
            with tc.tile_pool(name="ps_st2", bufs=1, space="PSUM") as ps_st2:
                _emit_ln(tc, ps_big2, ps_st2, pE2, y2, g2_sb, beta2_sb, yf)
            with tc.tile_pool(name="ps_tp2", bufs=2, space="PSUM") as ps_tp2:
                for t in range(TCH):
                    on = pE2.tile([128, D], F32, tag="on", name="on")
                    for j in range(DCH):
                        tp = ps_tp2.tile([128, 128], F32, tag="tp", name="tp")
                        nc.tensor.transpose(tp[:],
                                            yf[:, j, t * 128 : (t + 1) * 128],
                                            ident[:])
                        nc.vector.tensor_copy(on[:, j * 128 : (j + 1) * 128],
                                              tp[:])
                    nc.sync.dma_start(out_ap[t * 128 : (t + 1) * 128, :], on[:])


_CACHE = {}


def _build():
    if "nc" in _CACHE:
        return _CACHE["nc"]
    nc = bacc.Bacc("TRN2", target_bir_lowering=False, debug=False,
                   num_devices=NCORES)
    t_in = {}
    t_in["x_shard"] = nc.dram_tensor("x_shard", [TOK, D], F32,
                                     kind="ExternalInput").ap()
    for name, shape in (
        ("wq", [D, D]), ("bq", [D]), ("wk", [D, D]), ("bk", [D]),
        ("wv", [D, D]), ("bv", [D]), ("wo", [D, D]), ("bo", [D]),
        ("w1", [DFF, D]), ("b1", [DFF]), ("w2", [D, DFF]), ("b2", [D]),
        ("g1", [D]), ("beta1", [D]), ("g2", [D]), ("beta2", [D]),
    ):
        t_in[name] = nc.dram_tensor(name, shape, F32, kind="ExternalInput").ap()
    t_out = {"out_shard": nc.dram_tensor("out_shard", [TOK, D], F32,
                                         kind="ExternalOutput").ap()}
    with tile.TileContext(nc) as tc:
        _emit_body(tc, t_in, t_out)
    nc.compile()
    _CACHE["nc"] = nc
    return nc


def _in_maps(inputs):
    f = lambda k: np.ascontiguousarray(np.asarray(inputs[k], dtype=np.float32))
    x = f("x")
    shared = {k: f(k) for k in inputs if k != "x"}
    maps = []
    for core in range(NCORES):
        g, r = divmod(core, GROUP)
        m = dict(shared)
        m["x_shard"] = np.ascontiguousarray(x[g, r * TOK : (r + 1) * TOK, :])
        maps.append(m)
    return maps


def kernel(**inputs):
    nc = _build()
    maps = _in_maps(inputs)
    res = run_bass_kernel_spmd(nc, maps, core_ids=list(range(NCORES)))
    shards = [res.results[i]["out_shard"] for i in range(NCORES)]
    out = np.concatenate(shards, axis=0).reshape(B, S, D)
    return out.astype(np.float32)
